# revision 1
# baseline (speedup 1.0000x reference)
"""Trainium2 Bass kernel for nn_AttentionLayer (B=4, N=4096, D=1024).

Reference computation:
  nx = layernorm(x)
  h  = nx @ expand                       # [B,N,4352]
  q  = h[:, :128] ; k = h[:, 128:256]
  linear = h[:, 256:2304]; pre_gelu = h[:, 2304:4352]
  gated  = linear * gelu(pre_gelu)       # exact erf gelu
  local  = gated[:, :1024]; v = gated[:, 1024:2048]
  mask[i,j] = j<=i ? sigmoid((j-i)+pbm) : -inf
  attn = softmax(q k^T / sqrt(128) + mask) @ v
  out  = x + concat([local, attn]) @ project

Sharding (8 cores, SPMD): batch b -> core pair (2b, 2b+1).  Per pair,
512-row query blocks interleave for causal load balance: even core owns
blocks {0,3,4,7}, odd owns {1,2,5,6}.  Each core computes LN + expand for
its OWN 2048 rows only; k/v of the other half arrive via four pairwise
AllGathers (one per 512-row chunk, issued as each chunk's k/v lands in
HBM so the wire time pipelines under the remaining expand).  The kv slot
order is the fixed pair order [even-core blocks | odd-core blocks], the
same on both cores, so the SPMD attention schedule is uniform:
q-slot i attends a fixed slot set (2/4/6/8 slots), and causality +
position bias are enforced by a host-precomputed multiplicative mask
expM = causal ? exp(sigmoid(j-i+pbm)) : 0; the device computes
P = exp(qk) * expM and normalizes by the row sum (no max subtraction:
logits are O(1) after layernorm + xavier weights).  LN stats/centering
are software-pipelined one chunk ahead of the expand matmuls.

All matmuls in bf16 (fp32 matmul is 4x slower on TRN2), psum accumulation f32.
"""

import math

import numpy as np
import ml_dtypes

import concourse.bass as bass
import concourse.mybir as mybir
from concourse import bacc
import concourse.tile as tile
from concourse.bass_utils import run_bass_kernel_spmd

BF16 = mybir.dt.bfloat16
F32 = mybir.dt.float32
AF = mybir.ActivationFunctionType

B, N, D = 4, 4096, 1024
QK = 128
E = 2048
NB = 1024          # query/key block
R = N              # kv rows per core
RO = 2048          # own query rows per core
DCH = D // 128     # 8 contraction chunks
NT = 512           # matmul free-dim tile
W2 = 2176          # 128 (q or k) + 1024 (linear) + 1024 (gelu) cols

LAST_RESULTS = None  # set by kernel(); test harness reads exec_time_ns


def _build_nc(trace_friendly_names=True):
    nc = bacc.Bacc(None)

    xt = nc.declare_dram_parameter("xt", [D, RO], BF16, isOutput=False)
    xo = nc.declare_dram_parameter("xo", [RO, D], F32, isOutput=False)
    wkv = nc.declare_dram_parameter("wkv", [D, W2], BF16, isOutput=False)
    wql = nc.declare_dram_parameter("wql", [D, W2], BF16, isOutput=False)
    wproj = nc.declare_dram_parameter("wproj", [E, D], BF16, isOutput=False)
    msk = nc.declare_dram_parameter("msk", [10240, NT], BF16, isOutput=False)
    out = nc.declare_dram_parameter("out", [RO, D], F32, isOutput=True)

    RCH = R // NT           # 8 row chunks of 512
    ROCH = RO // NT         # 4 own row chunks

    with tile.TileContext(nc) as tc:
        with tc.tile_pool(name="const", bufs=1) as cpool:
            ones128 = cpool.tile([128, 1], BF16)
            nc.vector.memset(ones128[:], 1.0)
            ones1 = cpool.tile([1, 128], BF16)
            nc.vector.memset(ones1[:], 1.0)

            with tc.tile_pool(name="dram", bufs=1, space="DRAM") as dpool:
                kv_own = [dpool.tile([128, 4 * 1024 + (RO if r == 3 else 0)], BF16,
                                     name=f"kv_own_{r}") for r in range(4)]
                kv_all = [dpool.tile([2 * 128, 4 * 1024 + (RO if r == 3 else 0)], BF16,
                                     name=f"kv_all_{r}") for r in range(4)]

                with tc.tile_pool(name="persist", bufs=1) as ppool:
                    kT_sb = ppool.tile([128, R], BF16)         # k^T, hT layout
                    qT_sb = ppool.tile([128, RO], BF16)        # q^T (prescaled 1/sqrt(qk))
                    localT_sb = ppool.tile([128, 8 * RO], BF16)  # [lc_ch][128, 2048]

                    # ---------------- Phase 1+2: expand ----------------
                    with tc.tile_pool(name="wkv_p", bufs=1) as wkvp, \
                         tc.tile_pool(name="wql_p", bufs=1) as wqlp, \
                         tc.tile_pool(name="ex_stream", bufs=4) as estream, \
                         tc.tile_pool(name="ex_work", bufs=3) as ework, \
                         tc.tile_pool(name="st_work", bufs=2) as swork, \
                         tc.tile_pool(name="ex_psum", bufs=5, space="PSUM") as epsum, \
                         tc.tile_pool(name="st_psum", bufs=2, space="PSUM") as spsum:
                        wkv_sb = wkvp.tile([128, DCH * W2], BF16)
                        for dch in range(DCH):
                            nc.sync.dma_start(wkv_sb[:, dch * W2:(dch + 1) * W2],
                                              wkv[dch * 128:(dch + 1) * 128, :])
                        wql_sb = wqlp.tile([128, DCH * W2], BF16)
                        for dch in range(DCH):
                            nc.sync.dma_start(wql_sb[:, dch * W2:(dch + 1) * W2],
                                              wql[dch * 128:(dch + 1) * 128, :])

                        def stats_chain(rch):
                            """DMA x^T tiles for rch and compute LN scale/shift
                            broadcast tiles.  Emitted one iteration ahead so the
                            DVE math hides under the previous chunk's expand."""
                            rs = rch * NT
                            xts = []
                            for dch in range(DCH):
                                t = estream.tile([128, NT], BF16, tag="xt_e", bufs=10,
                                                 name=f"xt_{rch}_{dch}")
                                nc.sync.dma_start(t[:], xt[dch * 128:(dch + 1) * 128, rs:rs + NT])
                                xts.append(t)
                            mu_ps = spsum.tile([1, NT], F32, tag="stat", name=f"mu_ps_{rch}")
                            sq_ps = spsum.tile([1, NT], F32, tag="stat", name=f"sq_ps_{rch}")
                            # accumulate the 8 d-chunks on DVE (bf16 2x mode), then a
                            # single partition-sum matmul per stat instead of 8 each
                            acc_mu = estream.tile([128, NT], BF16, tag="acc_mu", bufs=3,
                                                  name=f"accmu_{rch}")
                            acc_sq = estream.tile([128, NT], BF16, tag="acc_sq", bufs=3,
                                                  name=f"accsq_{rch}")
                            sq_prev = estream.tile([128, NT], BF16, tag="sq_s", bufs=3,
                                                    name=f"sq_{rch}_0")
                            nc.scalar.activation(sq_prev[:], xts[0][:], AF.Square)
                            nc.vector.tensor_add(acc_mu[:], xts[0][:], xts[1][:])
                            for dch in range(1, DCH):
                                sqt = estream.tile([128, NT], BF16, tag="sq_s", bufs=3,
                                                   name=f"sq_{rch}_{dch}")
                                nc.scalar.activation(sqt[:], xts[dch][:], AF.Square)
                                if dch == 1:
                                    nc.vector.tensor_add(acc_sq[:], sq_prev[:], sqt[:])
                                else:
                                    nc.vector.tensor_add(acc_sq[:], acc_sq[:], sqt[:])
                                if dch >= 2:
                                    nc.vector.tensor_add(acc_mu[:], acc_mu[:], xts[dch][:])
                            nc.tensor.matmul(mu_ps[:], ones128[:], acc_mu[:],
                                             start=True, stop=True)
                            nc.tensor.matmul(sq_ps[:], ones128[:], acc_sq[:],
                                             start=True, stop=True)
                            mu = swork.tile([1, NT], F32, tag="st_mu", bufs=1, name=f"mu_{rch}")
                            e2 = swork.tile([1, NT], F32, tag="st_e2", bufs=1, name=f"e2_{rch}")
                            scr = swork.tile([1, NT], F32, tag="st_scr", bufs=1, name=f"scr_{rch}")
                            nc.vector.tensor_scalar_mul(mu[:], mu_ps[:], 1.0 / D)
                            nc.vector.tensor_scalar_mul(e2[:], sq_ps[:], 1.0 / D)
                            nc.vector.tensor_mul(scr[:], mu[:], mu[:])
                            nc.vector.tensor_sub(e2[:], e2[:], scr[:])
                            nc.vector.tensor_scalar_add(e2[:], e2[:], 1e-5)
                            nc.scalar.activation(e2[:], e2[:], AF.Sqrt)
                            nc.vector.reciprocal_approx_fast(scr[:], e2[:])          # rstd
                            nc.vector.scalar_tensor_tensor(
                                mu[:], mu[:], -1.0, scr[:],
                                op0=mybir.AluOpType.mult, op1=mybir.AluOpType.mult)  # -mu*rstd
                            rstd16 = swork.tile([1, NT], BF16, tag="st_r16", name=f"r16_{rch}")
                            sneg16 = swork.tile([1, NT], BF16, tag="st_s16", name=f"s16_{rch}")
                            nc.vector.tensor_copy(rstd16[:], scr[:])
                            nc.vector.tensor_copy(sneg16[:], mu[:])
                            return xts, rstd16, sneg16

                        def bcast_chain(rch, rstd16, sneg16):
                            # rank-1 broadcast [1,NT] -> [128,NT]; emitted mid-expand
                            # of the previous chunk so the DVE math above is hidden
                            bps = spsum.tile([128, NT], F32, tag="bcast", bufs=1, name=f"bps_{rch}")
                            nc.tensor.matmul(bps[:], ones1[:], rstd16[:], start=True, stop=True)
                            rstd_bt = swork.tile([128, NT], BF16, tag="rbt", bufs=3,
                                                 name=f"rbt_{rch}")
                            nc.vector.tensor_copy(rstd_bt[:], bps[:])
                            bps2 = spsum.tile([128, NT], F32, tag="bcast", bufs=1, name=f"bps2_{rch}")
                            nc.tensor.matmul(bps2[:], ones1[:], sneg16[:], start=True, stop=True)
                            sneg_bt = swork.tile([128, NT], BF16, tag="sbt", bufs=3,
                                                 name=f"sbt_{rch}")
                            nc.vector.tensor_copy(sneg_bt[:], bps2[:])
                            return rstd_bt, sneg_bt

                        # own chunks (0-3, ~70us expand each) interleaved with
                        # foreign chunks (4-7, ~30us) so every next-chunk stats
                        # chain has a long expand to hide under
                        rch_order = [0, 1, 2, 3]

                        def center_chain(rch, xts, rstd_bt, sneg_bt):
                            # x'' = x*rstd - mu*rstd; emitted mid-way through the
                            # PREVIOUS chunk's expand so the DVE work is hidden
                            xpp = []
                            for dch in range(DCH):
                                xc = estream.tile([128, NT], BF16, tag="xpp", bufs=34,
                                                  name=f"xpp_{rch}_{dch}")
                                nc.vector.tensor_mul(xc[:], xts[dch][:], rstd_bt[:])
                                nc.vector.tensor_add(xc[:], xc[:], sneg_bt[:])
                                xpp.append(xc)
                            return xpp

                        def v_group(rch, xpp, ms):
                            for m in ms:
                                vlin = ework.tile([128, E // 2], BF16, tag="vlin")
                                vgel = ework.tile([128, E // 2], BF16, tag="vgel")
                                for vc in range(4):
                                    vps = epsum.tile([128, NT], F32, tag="mm")
                                    if vc < 2:
                                        woff = 128 + vc * NT
                                    else:
                                        woff = 1152 + (vc - 2) * NT
                                    for dch in range(DCH):
                                        nc.tensor.matmul(
                                            vps[:],
                                            xpp[dch][:, m * 128:(m + 1) * 128],
                                            wkv_sb[:, dch * W2 + woff:dch * W2 + woff + NT],
                                            start=(dch == 0), stop=(dch == DCH - 1))
                                    if vc < 2:
                                        nc.vector.tensor_copy(vlin[:, vc * NT:(vc + 1) * NT], vps[:])
                                    else:
                                        nc.scalar.activation(vgel[:, (vc - 2) * NT:(vc - 1) * NT],
                                                             vps[:], AF.Gelu)
                                vv = ework.tile([128, E // 2], BF16, tag="vv")
                                nc.vector.tensor_mul(vv[:], vlin[:], vgel[:])
                                nc.sync.dma_start(kv_own[rch][:, m * 1024:(m + 1) * 1024], vv[:])

                        st0 = stats_chain(rch_order[0])
                        bt0 = bcast_chain(rch_order[0], st0[1], st0[2])
                        xpp_stash = center_chain(rch_order[0], st0[0], bt0[0], bt0[1])
                        xpps = {}
                        NOWN = 4
                        for oi, rch in enumerate(rch_order):
                            rs = rch * NT
                            xpp = xpp_stash
                            xpps[rch] = xpp
                            if oi + 1 < NOWN:
                                nxt = stats_chain(rch_order[oi + 1])
                            # k^T own (hT layout) -> DRAM bounce for the AllGather
                            kps = epsum.tile([128, NT], F32, tag="mm")
                            for dch in range(DCH):
                                nc.tensor.matmul(kps[:], wkv_sb[:, dch * W2:dch * W2 + 128],
                                                 xpp[dch][:],
                                                 start=(dch == 0), stop=(dch == DCH - 1))
                            kout = ework.tile([128, NT], BF16, tag="kout", bufs=2,
                                              name=f"kout_{rch}")
                            nc.vector.tensor_copy(kout[:], kps[:])
                            nc.sync.dma_start(kv_own[3][:, 4096 + rs:4096 + rs + NT], kout[:])
                            v_group(rch, xpp, (0, 1))
                            if oi + 1 < NOWN:
                                nbt = bcast_chain(rch_order[oi + 1], nxt[1], nxt[2])
                                xpp_stash = center_chain(rch_order[oi + 1], nxt[0],
                                                         nbt[0], nbt[1])
                            v_group(rch, xpp, (2, 3))
                            # AllGather for this chunk's v (and, for the last, all k):
                            # issued as soon as the chunk is in HBM so the wire time
                            # pipelines under the remaining expand
                            nc.gpsimd.collective_compute(
                                "AllGather",
                                mybir.AluOpType.bypass,
                                replica_groups=[[0, 1], [2, 3], [4, 5], [6, 7]],
                                ins=[kv_own[rch].opt()],
                                outs=[kv_all[rch].opt()],
                            )
                        for r in range(2):
                            nc.sync.dma_start(
                                kT_sb[:, r * RO:(r + 1) * RO],
                                kv_all[3][r * 128:(r + 1) * 128, 4096:4096 + RO])
                        # loop2: q + local expand (covers the AllGather latency)
                        for rch in rch_order:
                            rs = rch * NT
                            xpp = xpps[rch]
                            qps = epsum.tile([128, NT], F32, tag="mm")
                            for dch in range(DCH):
                                nc.tensor.matmul(qps[:], wql_sb[:, dch * W2:dch * W2 + 128],
                                                 xpp[dch][:],
                                                 start=(dch == 0), stop=(dch == DCH - 1))
                            nc.vector.tensor_copy(qT_sb[:, rs:rs + NT], qps[:])
                            for lc in range(8):
                                lps = epsum.tile([128, NT], F32, tag="mm")
                                gps = epsum.tile([128, NT], F32, tag="mm")
                                for dch in range(DCH):
                                    nc.tensor.matmul(
                                        lps[:],
                                        wql_sb[:, dch * W2 + 128 + lc * 128:dch * W2 + 256 + lc * 128],
                                        xpp[dch][:],
                                        start=(dch == 0), stop=(dch == DCH - 1))
                                for dch in range(DCH):
                                    nc.tensor.matmul(
                                        gps[:],
                                        wql_sb[:, dch * W2 + 1152 + lc * 128:dch * W2 + 1280 + lc * 128],
                                        xpp[dch][:],
                                        start=(dch == 0), stop=(dch == DCH - 1))
                                lgel = ework.tile([128, NT], BF16, tag="lgel")
                                nc.scalar.activation(lgel[:], gps[:], AF.Gelu)
                                llin = ework.tile([128, NT], BF16, tag="llin")
                                nc.vector.tensor_copy(llin[:], lps[:])
                                nc.vector.tensor_mul(
                                    localT_sb[:, lc * RO + rs:lc * RO + rs + NT],
                                    llin[:], lgel[:])

                    # ---------------- Phase 3: attention ----------------
                    # attnT pool encloses phase 4 too (read by project)
                    with tc.tile_pool(name="attnT_p", bufs=1) as apool:
                      attnT_sb = apool.tile([128, 8 * RO], BF16)  # [vc_ch][128, 2048]
                      proj_sb = apool.tile([128, 16 * D], BF16)   # prefetched during attention
                      for _cch in range(16):
                          nc.sync.dma_start(proj_sb[:, _cch * D:(_cch + 1) * D],
                                            wproj[_cch * 128:(_cch + 1) * 128, :])
                      with tc.tile_pool(name="psb_p", bufs=1) as psbp, \
                         tc.tile_pool(name="at_stream", bufs=6) as astream, \
                         tc.tile_pool(name="at_work", bufs=2) as awork, \
                         tc.tile_pool(name="at_psum", bufs=4, space="PSUM") as apsum, \
                         tc.tile_pool(name="av_psum", bufs=2, space="PSUM") as avpsum:

                        def attention(qi, kr_slots, moff):
                            qcol = qi * NT
                            nkr = len(kr_slots) * 4  # 128-row kr chunks
                            psb = psbp.tile([128, 32 * NT], BF16, tag="psb")
                            den_ps = apsum.tile([1, NT], F32, tag="den", bufs=1)
                            den_acc = awork.tile([128, NT], BF16, tag="den_acc", bufs=2,
                                                 name=f"den_acc_{qcol}")
                            for i, krs in enumerate(kr_slots):
                                for j in range(4):
                                    ti = i * 4 + j
                                    kr0 = krs * NT + j * 128
                                    mr0 = moff + i * NT + j * 128
                                    pt_ps = apsum.tile([128, NT], F32, tag="pt", bufs=3)
                                    nc.tensor.matmul(pt_ps[:], kT_sb[:, kr0:kr0 + 128],
                                                     qT_sb[:, qcol:qcol + NT],
                                                     start=True, stop=True)
                                    pe = awork.tile([128, NT], BF16, tag="pe", bufs=4)
                                    nc.scalar.activation(pe[:], pt_ps[:], AF.Exp)
                                    mt = astream.tile([128, NT], BF16, tag="mt", bufs=10)
                                    nc.sync.dma_start(mt[:], msk[mr0:mr0 + 128, :])
                                    nc.vector.tensor_mul(psb[:, ti * NT:(ti + 1) * NT], pe[:], mt[:])
                                    if ti == 0:
                                        nc.vector.tensor_copy(den_acc[:], psb[:, 0:NT])
                                    else:
                                        nc.vector.tensor_add(den_acc[:], den_acc[:],
                                                             psb[:, ti * NT:(ti + 1) * NT])
                            nc.tensor.matmul(den_ps[:], ones128[:], den_acc[:],
                                             start=True, stop=True)
                            # AV first: the denom reciprocal chain is emitted
                            # after the first AV group so it hides under PE work.
                            rd_b = None
                            for g in range(2):
                                avs = [avpsum.tile([128, NT], F32, tag="av", bufs=4,
                                                   name=f"av{g}_{_i}")
                                       for _i in range(4)]
                                for i, krs in enumerate(kr_slots):
                                    for j in range(4):
                                        ti = i * 4 + j
                                        rb = krs * 4 + j
                                        vt = astream.tile([128, NT], BF16, tag="vt", bufs=10)
                                        gslot, vj = rb // 4, rb % 4
                                        vrank, vbuf = (0, gslot) if gslot < 4 else (1, gslot - 4)
                                        nc.sync.dma_start(
                                            vt[:], kv_all[vbuf][vrank * 128:(vrank + 1) * 128,
                                                               vj * 1024 + g * NT:vj * 1024 + (g + 1) * NT])
                                        for v4 in range(4):
                                            nc.tensor.matmul(avs[v4][:],
                                                             vt[:, v4 * 128:(v4 + 1) * 128],
                                                             psb[:, ti * NT:(ti + 1) * NT],
                                                             start=(ti == 0), stop=(ti == nkr - 1))
                                if g == 0:
                                    den = awork.tile([1, NT], F32, tag="den_sb")
                                    rec = awork.tile([1, NT], F32, tag="rec")
                                    rec16 = awork.tile([1, NT], BF16, tag="rec16")
                                    nc.vector.tensor_copy(den[:], den_ps[:])
                                    nc.vector.reciprocal_approx_fast(rec[:], den[:])
                                    nc.vector.tensor_copy(rec16[:], rec[:])
                                    rb_ps = apsum.tile([128, NT], F32, tag="pt", bufs=3)
                                    nc.tensor.matmul(rb_ps[:], ones1[:], rec16[:],
                                                     start=True, stop=True)
                                    rd_b = awork.tile([128, NT], BF16, tag="rd_b")
                                    nc.vector.tensor_copy(rd_b[:], rb_ps[:])
                                for v4 in range(4):
                                    vcch = g * 4 + v4
                                    nc.vector.tensor_mul(
                                        attnT_sb[:, vcch * RO + qcol:vcch * RO + qcol + NT],
                                        avs[v4][:], rd_b[:])

                        SCHED = {0: [0, 4], 1: [0, 1, 4, 5],
                                 2: [0, 1, 2, 4, 5, 6], 3: [0, 1, 2, 3, 4, 5, 6, 7]}
                        MOFF = {0: 0, 1: 1024, 2: 3072, 3: 6144}
                        for qi in range(4):
                            attention(qi, SCHED[qi], MOFF[qi])

                      # ---------------- Phase 4: project + residual ----------------
                      with tc.tile_pool(name="pr_stream", bufs=4) as prstream, \
                           tc.tile_pool(name="pr_psum", bufs=4, space="PSUM") as prpsum:
                          for rt in range(RO // 128):
                              for dc in range(2):
                                  ops = prpsum.tile([128, NT], F32, tag="out")
                                  for cch in range(16):
                                      if cch < 8:
                                          lhsT = localT_sb[:, cch * RO + rt * 128:cch * RO + (rt + 1) * 128]
                                      else:
                                          lhsT = attnT_sb[:, (cch - 8) * RO + rt * 128:(cch - 8) * RO + (rt + 1) * 128]
                                      nc.tensor.matmul(ops[:], lhsT,
                                                       proj_sb[:, cch * D + dc * NT:cch * D + (dc + 1) * NT],
                                                       start=(cch == 0), stop=(cch == 15))
                                  xo_t = prstream.tile([128, NT], F32, tag="xo")
                                  nc.sync.dma_start(xo_t[:], xo[rt * 128:(rt + 1) * 128, dc * NT:(dc + 1) * NT])
                                  ot = prstream.tile([128, NT], F32, tag="ot")
                                  nc.vector.tensor_add(ot[:], ops[:], xo_t[:])
                                  nc.sync.dma_start(out[rt * 128:(rt + 1) * 128, dc * NT:(dc + 1) * NT], ot[:])

    nc.compile()
    return nc


_ORDERS = {0: [0, 3, 4, 7, 1, 2, 5, 6], 1: [1, 2, 5, 6, 0, 3, 4, 7]}


def _sigmoid(x):
    return np.where(x >= 0, 1.0 / (1.0 + np.exp(-np.abs(x))),
                    np.exp(-np.abs(x)) / (1.0 + np.exp(-np.abs(x))))


def _prep_inputs(x, expand, project, pbm):
    """Build per-core input maps (host-side sharding)."""
    bf16 = ml_dtypes.bfloat16
    sc = 1.0 / math.sqrt(QK)
    wq = (expand[:, :QK] * sc)
    wk = expand[:, QK:2 * QK]
    lin = expand[:, 2 * QK:2 * QK + E]
    gel = expand[:, 2 * QK + E:]
    wkv = np.concatenate([wk, lin[:, D:], gel[:, D:]], axis=1).astype(bf16)
    wql = np.concatenate([wq, lin[:, :D], gel[:, :D]], axis=1).astype(bf16)
    wproj = project.astype(bf16)

    in_maps = []
    SCHED = {0: [0, 4], 1: [0, 1, 4, 5], 2: [0, 1, 2, 4, 5, 6], 3: [0, 1, 2, 3, 4, 5, 6, 7]}
    NBQ = 512
    for c in range(8):
        b, half = c // 2, c % 2
        order = _ORDERS[half]
        xb = x[b]
        xperm = np.concatenate([xb[blk * NBQ:(blk + 1) * NBQ] for blk in order[:4]], axis=0)
        xt = np.ascontiguousarray(xperm.T).astype(bf16)          # [1024, 2048] own rows only
        xo = np.ascontiguousarray(xperm).astype(np.float32)
        gq_all = np.concatenate([np.arange(blk * NBQ, (blk + 1) * NBQ) for blk in order[:4]]).astype(np.float64)
        # kv slots in FIXED pair order: [A blocks 0,3,4,7 | B blocks 1,2,5,6]
        kv_order = _ORDERS[0][:4] + _ORDERS[1][:4]
        gk_all = np.concatenate([np.arange(blk * NBQ, (blk + 1) * NBQ) for blk in kv_order]).astype(np.float64)

        def expM(gk_sub, gq_sub):
            diff = gk_sub[:, None] - gq_sub[None, :]
            m = np.where(diff <= 0, np.exp(_sigmoid(diff + pbm)), 0.0)
            return m.astype(bf16)

        parts = []
        for qi in range(4):
            gq = gq_all[qi * NBQ:(qi + 1) * NBQ]
            gk = np.concatenate([gk_all[s0 * NBQ:(s0 + 1) * NBQ] for s0 in SCHED[qi]])
            parts.append(expM(gk, gq))
        mskc = np.ascontiguousarray(np.concatenate(parts, axis=0))  # [10240, 512]
        in_maps.append({
            "xt": xt, "xo": xo, "wkv": wkv, "wql": wql, "wproj": wproj,
            "msk": mskc,
        })
    return in_maps


def kernel(x, expand, project, position_bias_mult):
    global LAST_RESULTS
    x = np.asarray(x, dtype=np.float32)
    expand = np.asarray(expand, dtype=np.float32)
    project = np.asarray(project, dtype=np.float32)
    pbm = float(np.asarray(position_bias_mult))

    in_maps = _prep_inputs(x, expand, project, pbm)
    nc = _build_nc()
    res = run_bass_kernel_spmd(nc, in_maps, core_ids=list(range(8)))
    LAST_RESULTS = res

    full = np.empty((B, N, D), dtype=np.float32)
    for c in range(8):
        b, half = c // 2, c % 2
        order = _ORDERS[half]
        o = res.results[c]["out"]
        for qi in range(4):
            blk = order[qi]
            full[b, blk * 512:(blk + 1) * 512] = o[qi * 512:(qi + 1) * 512]
    return full



# revision 9
# speedup vs baseline: 1.6905x; 1.6905x over previous
"""Trainium2 Bass kernel for nn_AttentionLayer (B=4, N=4096, D=1024) — v2 fp8.

Reference computation:
  nx = layernorm(x)
  h  = nx @ expand                       # [B,N,4352]
  q  = h[:, :128] ; k = h[:, 128:256]
  linear = h[:, 256:2304]; pre_gelu = h[:, 2304:4352]
  gated  = linear * gelu(pre_gelu)       # exact erf gelu
  local  = gated[:, :1024]; v = gated[:, 1024:2048]
  mask[i,j] = j<=i ? sigmoid((j-i)+pbm) : -inf
  attn = softmax(q k^T / sqrt(128) + mask) @ v
  out  = x + concat([local, attn]) @ project

Sharding (8 cores, SPMD): batch b -> core pair (2b, 2b+1); 512-row query
blocks interleave for causal balance (even core owns blocks {0,3,4,7}, odd
{1,2,5,6}).  Each core computes LN + expand for its OWN 2048 rows; k/v are
exchanged via four pairwise fp8 AllGathers (one per 512-row chunk).

v2 changes vs baseline:
  * all heavy matmuls in fp8 e4m3 with DoubleRow perf mode (2 K-chunks per
    pass): expand, local, q/k, AV, project.  Scores (K=128) in fp8 single
    rate.  Scale plan: W*64, x''*1, q*4, k*4, v*16, concat*16; descales are
    folded into ACT scale args and DVE scalar_tensor_tensor constants.
    Simulated end-to-end rel err 1.22e-2 (budget 2e-2).
  * V and K^T live in SBUF for the whole attention phase (loaded once per
    AllGather chunk) instead of being re-DMA'd per query block: removes the
    serialized per-tile DMA issue bottleneck on the sync engine.
  * mask multiply skipped for tiles where exp(sigmoid(j-i+pbm)) == 1 in
    bf16 on BOTH cores (union list keeps the SPMD stream uniform); masked
    tiles get a host-precomputed multiplicative expM tile.
  * batched DMAs (one per x chunk / weight tensor / mask buffer).
  * attention scores for q-block i+1 are emitted between the score and AV
    groups of block i so EXP latency hides under PE work.
"""

import math

import numpy as np
import ml_dtypes

import concourse.bass as bass
import concourse.mybir as mybir
from concourse import bacc
import concourse.tile as tile
from concourse.bass_utils import run_bass_kernel_spmd

BF16 = mybir.dt.bfloat16
FP8 = mybir.dt.float8e4
F32 = mybir.dt.float32
AF = mybir.ActivationFunctionType
DR = mybir.MatmulPerfMode.DoubleRow

B, N, D = 4, 4096, 1024
QK = 128
E = 2048
NT = 512           # row-chunk / matmul free-dim tile
RO = 2048          # own query rows per core
DCH = 8            # 128-deep contraction chunks in D
W2 = 2176          # 128 (q or k) + 1024 (lin half) + 1024 (gel half)

# scales (see fp8_sim2.py)
WS = 64.0          # weight scale for all fp8 weights
QS = 4.0           # q stored scale
KS = 4.0           # k stored scale
VS = 16.0          # v / gated stored scale
CS = 16.0          # concat (local/attn) stored scale == VS
S_E = 1.0 / WS                 # expand psum descale (xs=1)
S_L = 1.0 / (QS * KS * math.sqrt(QK))   # logits psum descale
S_OUT = 1.0 / (CS * WS)        # project psum descale

_QORD = {0: [0, 3, 4, 7], 1: [1, 2, 5, 6]}   # q-block of (half, qi)
KV_ORDER = [0, 3, 4, 7, 1, 2, 5, 6]          # global block of kv slot s
SCHED = {0: [0, 4], 1: [0, 1, 4, 5], 2: [0, 1, 2, 4, 5, 6],
         3: [0, 1, 2, 3, 4, 5, 6, 7]}

LAST_RESULTS = None  # set by kernel(); test harness reads exec_time_ns


def _build_sched(pbm):
    """Per qi: list of (slot, j, mask_idx|None).  mask needed iff some core's
    tile is not entirely in the exp(sigmoid)==1 (bf16) far-below-diag zone."""
    tiles = {}
    nm = 0
    for qi in range(4):
        lst = []
        for slot in SCHED[qi]:
            for j in range(4):
                kv_max = KV_ORDER[slot] * NT + j * 128 + 127
                masked = any(
                    not (kv_max - _QORD[h][qi] * NT + pbm < -6.5)
                    for h in (0, 1))
                if masked:
                    lst.append((slot, j, nm))
                    nm += 1
                else:
                    lst.append((slot, j, None))
        tiles[qi] = lst
    return tiles, nm


def _build_nc(tiles, nm):
    nc = bacc.Bacc(None)

    xt = nc.declare_dram_parameter("xt", [128, DCH, RO], BF16, isOutput=False)
    xo = nc.declare_dram_parameter("xo", [RO, D], F32, isOutput=False)
    wkv = nc.declare_dram_parameter("wkv", [128, DCH, W2], FP8, isOutput=False)
    wql = nc.declare_dram_parameter("wql", [128, DCH, W2], FP8, isOutput=False)
    wproj = nc.declare_dram_parameter("wproj", [128, 16, D], FP8, isOutput=False)
    msk = nc.declare_dram_parameter("msk", [128, max(nm, 1) * NT], BF16,
                                    isOutput=False)
    out = nc.declare_dram_parameter("out", [RO, D], F32, isOutput=True)

    with tile.TileContext(nc) as tc:
        with tc.tile_pool(name="const", bufs=1) as cpool:
            ones128 = cpool.tile([128, 1], BF16)
            nc.vector.memset(ones128[:], 1.0)
            ones1 = cpool.tile([1, 128], BF16)
            nc.vector.memset(ones1[:], 1.0)

            with tc.tile_pool(name="dram", bufs=1, space="DRAM") as dpool:
                kvp = [dpool.tile([128, 4608], FP8, name=f"kvp_{r}")
                       for r in range(4)]
                kv_all = [dpool.tile([2 * 128, 4608], FP8, name=f"kva_{r}")
                          for r in range(4)]

                with tc.tile_pool(name="persist", bufs=1) as ppool:
                    kT_sb = ppool.tile([128, 8, NT], FP8)       # k^T slots
                    qT_sb = ppool.tile([128, 4, NT], FP8)       # q^T chunks
                    v_sb = [ppool.tile([128, 4, 1024], FP8, name=f"vsb_{s}")
                            for s in range(8)]
                    localT_sb = ppool.tile([128, 8, RO], FP8)
                    attnT_sb = ppool.tile([128, 8, RO], FP8)

                    # ---------------- Phase 1: expand (kv part) ----------------
                    with tc.tile_pool(name="wkv_p", bufs=1) as wkvp, \
                         tc.tile_pool(name="wql_p", bufs=1) as wqlp, \
                         tc.tile_pool(name="ex_stream", bufs=2) as estream, \
                         tc.tile_pool(name="ex_work", bufs=3) as ework, \
                         tc.tile_pool(name="st_work", bufs=2) as swork, \
                         tc.tile_pool(name="ex_psum", bufs=5, space="PSUM") as epsum, \
                         tc.tile_pool(name="st_psum", bufs=2, space="PSUM") as spsum:
                        # x chunk 0 DMA first so stats can start immediately
                        xt_tiles = {}
                        x8_tiles = {}

                        def xt_dma(rch):
                            t = estream.tile([128, DCH, NT], BF16, tag="xt",
                                             bufs=2, name=f"xt_{rch}")
                            nc.sync.dma_start(t[:], xt[:, :, rch * NT:(rch + 1) * NT])
                            xt_tiles[rch] = t

                        xt_dma(0)
                        wkv_sb = wkvp.tile([128, DCH, W2], FP8)
                        nc.sync.dma_start(wkv_sb[:], wkv[:])
                        xt_dma(1)
                        wql_sb = wqlp.tile([128, DCH, W2], FP8)
                        nc.sync.dma_start(wql_sb[:], wql[:])

                        def stats_chain(rch):
                            xts = xt_tiles[rch]
                            mu_ps = spsum.tile([1, NT], F32, tag="stat",
                                               name=f"mu_ps_{rch}")
                            sq_ps = spsum.tile([1, NT], F32, tag="stat",
                                               name=f"sq_ps_{rch}")
                            acc_mu = ework.tile([128, NT], BF16, tag="acc_mu",
                                                bufs=3, name=f"accmu_{rch}")
                            acc_sq = ework.tile([128, NT], BF16, tag="acc_sq",
                                                bufs=3, name=f"accsq_{rch}")
                            sq_prev = ework.tile([128, NT], BF16, tag="sq_s",
                                                 bufs=3, name=f"sq_{rch}_0")
                            nc.scalar.activation(sq_prev[:], xts[:, 0:1, :], AF.Square)
                            nc.vector.tensor_add(acc_mu[:], xts[:, 0:1, :],
                                                 xts[:, 1:2, :])
                            for dch in range(1, DCH):
                                sqt = ework.tile([128, NT], BF16, tag="sq_s",
                                                 bufs=3, name=f"sq_{rch}_{dch}")
                                nc.scalar.activation(sqt[:], xts[:, dch:dch + 1, :],
                                                     AF.Square)
                                if dch == 1:
                                    nc.vector.tensor_add(acc_sq[:], sq_prev[:], sqt[:])
                                else:
                                    nc.vector.tensor_add(acc_sq[:], acc_sq[:], sqt[:])
                                if dch >= 2:
                                    nc.vector.tensor_add(acc_mu[:], acc_mu[:],
                                                         xts[:, dch:dch + 1, :])
                            nc.tensor.matmul(mu_ps[:], ones128[:], acc_mu[:],
                                             start=True, stop=True)
                            nc.tensor.matmul(sq_ps[:], ones128[:], acc_sq[:],
                                             start=True, stop=True)
                            mu = swork.tile([1, NT], F32, tag="st_mu", bufs=1,
                                            name=f"mu_{rch}")
                            e2 = swork.tile([1, NT], F32, tag="st_e2", bufs=1,
                                            name=f"e2_{rch}")
                            scr = swork.tile([1, NT], F32, tag="st_scr", bufs=1,
                                             name=f"scr_{rch}")
                            nc.vector.tensor_scalar_mul(mu[:], mu_ps[:], 1.0 / D)
                            nc.vector.tensor_scalar_mul(e2[:], sq_ps[:], 1.0 / D)
                            nc.vector.tensor_mul(scr[:], mu[:], mu[:])
                            nc.vector.tensor_sub(e2[:], e2[:], scr[:])
                            nc.vector.tensor_scalar_add(e2[:], e2[:], 1e-5)
                            nc.scalar.activation(e2[:], e2[:], AF.Sqrt)
                            nc.vector.reciprocal_approx_fast(scr[:], e2[:])   # rstd
                            nc.vector.scalar_tensor_tensor(
                                mu[:], mu[:], -1.0, scr[:],
                                op0=mybir.AluOpType.mult, op1=mybir.AluOpType.mult)
                            rstd16 = swork.tile([1, NT], BF16, tag="st_r16",
                                                name=f"r16_{rch}")
                            sneg16 = swork.tile([1, NT], BF16, tag="st_s16",
                                                name=f"s16_{rch}")
                            nc.vector.tensor_copy(rstd16[:], scr[:])
                            nc.vector.tensor_copy(sneg16[:], mu[:])
                            return rstd16, sneg16

                        def bcast_chain(rch, rstd16, sneg16):
                            bps = spsum.tile([128, NT], F32, tag="bcast", bufs=1,
                                             name=f"bps_{rch}")
                            nc.tensor.matmul(bps[:], ones1[:], rstd16[:],
                                             start=True, stop=True)
                            rstd_bt = swork.tile([128, NT], BF16, tag="rbt",
                                                 bufs=3, name=f"rbt_{rch}")
                            nc.vector.tensor_copy(rstd_bt[:], bps[:])
                            bps2 = spsum.tile([128, NT], F32, tag="bcast", bufs=1,
                                              name=f"bps2_{rch}")
                            nc.tensor.matmul(bps2[:], ones1[:], sneg16[:],
                                             start=True, stop=True)
                            sneg_bt = swork.tile([128, NT], BF16, tag="sbt",
                                                 bufs=3, name=f"sbt_{rch}")
                            nc.vector.tensor_copy(sneg_bt[:], bps2[:])
                            return rstd_bt, sneg_bt

                        def center_chain(rch, rstd_bt, sneg_bt):
                            xts = xt_tiles[rch]
                            x8 = estream.tile([128, DCH, NT], FP8, tag="x8",
                                              bufs=4, name=f"x8_{rch}")
                            for dch in range(DCH):
                                tmp = ework.tile([128, NT], BF16, tag="ctmp",
                                                 bufs=3, name=f"ct_{rch}_{dch}")
                                nc.vector.tensor_mul(tmp[:], xts[:, dch:dch + 1, :],
                                                     rstd_bt[:])
                                nc.vector.tensor_add(x8[:, dch:dch + 1, :],
                                                     tmp[:], sneg_bt[:])
                            x8_tiles[rch] = x8
                            return x8

                        def v_group(rch, x8, ms):
                            for m in ms:
                                vps = [epsum.tile([128, NT], F32, tag="mm",
                                                  name=f"vps_{rch}_{m}_{_i}")
                                       for _i in range(4)]
                                for pc in range(4):
                                    lhs = x8[:, 2 * pc:2 * pc + 2,
                                             m * 128:(m + 1) * 128]
                                    for vc in range(4):
                                        woff = 128 + vc * NT
                                        nc.tensor.matmul(
                                            vps[vc][:], lhs,
                                            wkv_sb[:, 2 * pc:2 * pc + 2,
                                                   woff:woff + NT],
                                            start=(pc == 0), stop=(pc == 3),
                                            perf_mode=DR)
                                vgel = ework.tile([128, 2 * NT], BF16, tag="vgel")
                                nc.scalar.activation(vgel[:, 0:NT], vps[2][:],
                                                     AF.Gelu, scale=S_E)
                                nc.scalar.activation(vgel[:, NT:2 * NT], vps[3][:],
                                                     AF.Gelu, scale=S_E)
                                vv = ework.tile([128, 2 * NT], FP8, tag="vv",
                                                bufs=3)
                                nc.vector.scalar_tensor_tensor(
                                    vv[:, 0:NT], vps[0][:], S_E * VS,
                                    vgel[:, 0:NT],
                                    op0=mybir.AluOpType.mult,
                                    op1=mybir.AluOpType.mult)
                                nc.vector.scalar_tensor_tensor(
                                    vv[:, NT:2 * NT], vps[1][:], S_E * VS,
                                    vgel[:, NT:2 * NT],
                                    op0=mybir.AluOpType.mult,
                                    op1=mybir.AluOpType.mult)
                                nc.sync.dma_start(
                                    kvp[rch][:, m * 1024:(m + 1) * 1024], vv[:])

                        # chunk pipeline: stats one chunk ahead
                        st0 = stats_chain(0)
                        bt0 = bcast_chain(0, st0[0], st0[1])
                        x8_stash = center_chain(0, bt0[0], bt0[1])
                        for rch in range(4):
                            if rch + 2 <= 3:
                                xt_dma(rch + 2)
                            x8 = x8_stash
                            # k^T (hT layout) -> DRAM for the AllGather
                            kps = epsum.tile([128, NT], F32, tag="mm")
                            for pc in range(4):
                                nc.tensor.matmul(kps[:],
                                                 wkv_sb[:, 2 * pc:2 * pc + 2, 0:128],
                                                 x8[:, 2 * pc:2 * pc + 2, :],
                                                 start=(pc == 0), stop=(pc == 3),
                                                 perf_mode=DR)
                            kout = ework.tile([128, NT], FP8, tag="kout", bufs=2,
                                              name=f"kout_{rch}")
                            nc.vector.tensor_scalar_mul(kout[:], kps[:], S_E * KS)
                            nc.sync.dma_start(kvp[rch][:, 4096:4608], kout[:])
                            v_group(rch, x8, (0, 1))
                            if rch + 1 <= 3:
                                nxt = stats_chain(rch + 1)
                                nbt = bcast_chain(rch + 1, nxt[0], nxt[1])
                                x8_stash = center_chain(rch + 1, nbt[0], nbt[1])
                            v_group(rch, x8, (2, 3))
                            nc.gpsimd.collective_compute(
                                "AllGather",
                                mybir.AluOpType.bypass,
                                replica_groups=[[0, 1], [2, 3], [4, 5], [6, 7]],
                                ins=[kvp[rch].opt()],
                                outs=[kv_all[rch].opt()],
                            )
                            # slot loads (own + foreign) for attention
                            for rank in range(2):
                                slot = rank * 4 + rch
                                nc.sync.dma_start(
                                    v_sb[slot][:],
                                    kv_all[rch][rank * 128:(rank + 1) * 128, 0:4096])
                                nc.sync.dma_start(
                                    kT_sb[:, slot:slot + 1, :],
                                    kv_all[rch][rank * 128:(rank + 1) * 128,
                                                4096:4608])

                        # ---------------- Phase 2: q + local expand ----------------
                        for rch in range(4):
                            x8 = x8_tiles[rch]
                            qps = epsum.tile([128, NT], F32, tag="mm")
                            for pc in range(4):
                                nc.tensor.matmul(qps[:],
                                                 wql_sb[:, 2 * pc:2 * pc + 2, 0:128],
                                                 x8[:, 2 * pc:2 * pc + 2, :],
                                                 start=(pc == 0), stop=(pc == 3),
                                                 perf_mode=DR)
                            nc.vector.tensor_scalar_mul(
                                qT_sb[:, rch:rch + 1, :], qps[:], S_E * QS)
                            for lc in range(8):
                                lps = epsum.tile([128, NT], F32, tag="mm")
                                gps = epsum.tile([128, NT], F32, tag="mm")
                                for pc in range(4):
                                    nc.tensor.matmul(
                                        lps[:],
                                        wql_sb[:, 2 * pc:2 * pc + 2,
                                               128 + lc * 128:256 + lc * 128],
                                        x8[:, 2 * pc:2 * pc + 2, :],
                                        start=(pc == 0), stop=(pc == 3),
                                        perf_mode=DR)
                                for pc in range(4):
                                    nc.tensor.matmul(
                                        gps[:],
                                        wql_sb[:, 2 * pc:2 * pc + 2,
                                               1152 + lc * 128:1280 + lc * 128],
                                        x8[:, 2 * pc:2 * pc + 2, :],
                                        start=(pc == 0), stop=(pc == 3),
                                        perf_mode=DR)
                                lgel = ework.tile([128, NT], BF16, tag="lgel")
                                nc.scalar.activation(lgel[:], gps[:], AF.Gelu,
                                                     scale=S_E)
                                nc.vector.scalar_tensor_tensor(
                                    localT_sb[:, lc:lc + 1,
                                              rch * NT:(rch + 1) * NT],
                                    lps[:], S_E * CS, lgel[:],
                                    op0=mybir.AluOpType.mult,
                                    op1=mybir.AluOpType.mult)

    # ---------------- Phase 3: attention ----------------
                    with tc.tile_pool(name="proj_p", bufs=1) as projp:
                      proj_sb = projp.tile([128, 16, D], FP8)
                      msk_sb = projp.tile([128, max(nm, 1), NT], BF16)
                      # split mask prefetch: first qi-0 tiles, then the rest
                      sp = min(8, nm) if nm else 0
                      if sp:
                          nc.sync.dma_start(msk_sb[:, 0:sp, :], msk[:, 0:sp * NT])
                      nc.sync.dma_start(proj_sb[:], wproj[:])
                      if nm > sp:
                          nc.sync.dma_start(msk_sb[:, sp:nm, :],
                                            msk[:, sp * NT:nm * NT])
                      with tc.tile_pool(name="psb_p", bufs=2) as psbp, \
                           tc.tile_pool(name="at_work", bufs=2) as awork, \
                           tc.tile_pool(name="at_psum", bufs=3, space="PSUM") as apsum, \
                           tc.tile_pool(name="av_psum", bufs=4, space="PSUM") as avpsum, \
                           tc.tile_pool(name="dn_psum", bufs=1, space="PSUM") as dnpsum:

                        psbs = {}
                        dens = {}

                        def scores_block(qi):
                            tl = tiles[qi]
                            nkr = len(tl)
                            psb = psbp.tile([128, 32, NT], FP8, tag="psb",
                                            name=f"psb_{qi % 2}")
                            den_acc = awork.tile([128, NT], BF16, tag="den_acc",
                                                 bufs=2, name=f"den_acc_{qi}")
                            for ti, (slot, j, mi) in enumerate(tl):
                                pt_ps = apsum.tile([128, NT], F32, tag="pt",
                                                   bufs=3)
                                nc.tensor.matmul(
                                    pt_ps[:],
                                    kT_sb[:, slot:slot + 1, j * 128:(j + 1) * 128],
                                    qT_sb[:, qi:qi + 1, :],
                                    start=True, stop=True)
                                if mi is None:
                                    nc.scalar.activation(psb[:, ti:ti + 1, :],
                                                         pt_ps[:], AF.Exp,
                                                         scale=S_L)
                                    if ti == 0:
                                        nc.vector.tensor_copy(den_acc[:],
                                                              psb[:, 0:1, :])
                                    else:
                                        nc.vector.tensor_add(den_acc[:], den_acc[:],
                                                             psb[:, ti:ti + 1, :])
                                else:
                                    pe = awork.tile([128, NT], BF16, tag="pe",
                                                    bufs=4)
                                    nc.scalar.activation(pe[:], pt_ps[:], AF.Exp,
                                                         scale=S_L)
                                    nc.vector.tensor_mul(
                                        psb[:, ti:ti + 1, :], pe[:],
                                        msk_sb[:, mi:mi + 1, :])
                                    if ti == 0:
                                        nc.vector.tensor_copy(den_acc[:],
                                                              psb[:, 0:1, :])
                                    else:
                                        nc.vector.tensor_add(den_acc[:], den_acc[:],
                                                             psb[:, ti:ti + 1, :])
                            psbs[qi] = psb
                            dens[qi] = den_acc

                        def av_block(qi):
                            tl = tiles[qi]
                            nkr = len(tl)
                            psb = psbs[qi]
                            den_ps = dnpsum.tile([1, NT], F32, tag="den", bufs=1)
                            nc.tensor.matmul(den_ps[:], ones128[:], dens[qi][:],
                                             start=True, stop=True)
                            rd_b = None
                            for g in range(2):
                                avs = [avpsum.tile([128, NT], F32, tag="av",
                                                   bufs=4, name=f"av{qi}_{g}_{i}")
                                       for i in range(4)]
                                for tp in range(nkr // 2):
                                    slot, j, _ = tl[2 * tp]
                                    jp = j // 2
                                    for v4 in range(4):
                                        nc.tensor.matmul(
                                            avs[v4][:],
                                            v_sb[slot][:, 2 * jp:2 * jp + 2,
                                                       g * NT + v4 * 128:
                                                       g * NT + (v4 + 1) * 128],
                                            psb[:, 2 * tp:2 * tp + 2, :],
                                            start=(tp == 0),
                                            stop=(tp == nkr // 2 - 1),
                                            perf_mode=DR)
                                if g == 0:
                                    den = awork.tile([1, NT], F32, tag="den_sb")
                                    rec = awork.tile([1, NT], F32, tag="rec")
                                    rec16 = awork.tile([1, NT], BF16, tag="rec16")
                                    nc.vector.tensor_copy(den[:], den_ps[:])
                                    nc.vector.reciprocal_approx_fast(rec[:], den[:])
                                    nc.vector.tensor_copy(rec16[:], rec[:])
                                    rb_ps = apsum.tile([128, NT], F32, tag="pt",
                                                       bufs=3)
                                    nc.tensor.matmul(rb_ps[:], ones1[:], rec16[:],
                                                     start=True, stop=True)
                                    rd_b = awork.tile([128, NT], BF16, tag="rd_b")
                                    nc.vector.tensor_copy(rd_b[:], rb_ps[:])
                                for v4 in range(4):
                                    nc.vector.tensor_mul(
                                        attnT_sb[:, g * 4 + v4:g * 4 + v4 + 1,
                                                 qi * NT:(qi + 1) * NT],
                                        avs[v4][:], rd_b[:])

                        scores_block(0)
                        scores_block(1)
                        av_block(0)
                        scores_block(2)
                        av_block(1)
                        scores_block(3)
                        av_block(2)
                        av_block(3)

                      # ---------------- Phase 4: project + residual ----------------
                      with tc.tile_pool(name="pr_stream", bufs=4) as prstream, \
                           tc.tile_pool(name="pr_psum", bufs=4, space="PSUM") as prpsum:
                          for rt in range(RO // 128):
                              ops = [prpsum.tile([128, NT], F32, tag="out",
                                                 name=f"ops_{rt}_{_i}")
                                     for _i in range(2)]
                              for p in range(8):
                                  if p < 4:
                                      lhs = localT_sb[:, 2 * p:2 * p + 2,
                                                      rt * 128:(rt + 1) * 128]
                                  else:
                                      lhs = attnT_sb[:, 2 * (p - 4):2 * (p - 4) + 2,
                                                     rt * 128:(rt + 1) * 128]
                                  for dc in range(2):
                                      nc.tensor.matmul(
                                          ops[dc][:], lhs,
                                          proj_sb[:, 2 * p:2 * p + 2,
                                                  dc * NT:(dc + 1) * NT],
                                          start=(p == 0), stop=(p == 7),
                                          perf_mode=DR)
                              xo_t = prstream.tile([128, D], F32, tag="xo")
                              nc.sync.dma_start(xo_t[:],
                                                xo[rt * 128:(rt + 1) * 128, :])
                              ot = prstream.tile([128, D], F32, tag="ot")
                              for dc in range(2):
                                  nc.vector.scalar_tensor_tensor(
                                      ot[:, dc * NT:(dc + 1) * NT], ops[dc][:],
                                      S_OUT, xo_t[:, dc * NT:(dc + 1) * NT],
                                      op0=mybir.AluOpType.mult,
                                      op1=mybir.AluOpType.add)
                              nc.sync.dma_start(out[rt * 128:(rt + 1) * 128, :],
                                                ot[:])

    nc.compile()
    return nc


def _sigmoid(x):
    return np.where(x >= 0, 1.0 / (1.0 + np.exp(-np.abs(x))),
                    np.exp(-np.abs(x)) / (1.0 + np.exp(-np.abs(x))))


def _q8(x, scale):
    y = np.clip(np.asarray(x, np.float32) * scale, -240.0, 240.0)
    return np.asarray(y, dtype=ml_dtypes.float8_e4m3)


def _prep_inputs(x, expand, project, pbm, tiles, nm):
    """Build per-core input maps (host-side sharding)."""
    bf16 = ml_dtypes.bfloat16
    wq = expand[:, :QK]
    wk = expand[:, QK:2 * QK]
    lin = expand[:, 2 * QK:2 * QK + E]
    gel = expand[:, 2 * QK + E:]
    # [128, DCH, W2]: [p, dch, col] = W[dch*128+p, colmap]
    wkv_f = np.concatenate([wk, lin[:, D:], gel[:, D:]], axis=1)
    wql_f = np.concatenate([wq, lin[:, :D], gel[:, :D]], axis=1)
    wkv_h = _q8(wkv_f.reshape(DCH, 128, W2).transpose(1, 0, 2), WS)
    wql_h = _q8(wql_f.reshape(DCH, 128, W2).transpose(1, 0, 2), WS)
    wproj_h = _q8(project.reshape(16, 128, D).transpose(1, 0, 2), WS)

    in_maps = []
    for c in range(8):
        b, half = c // 2, c % 2
        order = _QORD[half]
        xb = x[b]
        xperm = np.concatenate([xb[blk * NT:(blk + 1) * NT] for blk in order],
                               axis=0)                      # [2048, 1024]
        # xt[p, dch, n] = xperm[n, dch*128+p]
        xt_h = np.ascontiguousarray(
            xperm.reshape(RO, DCH, 128).transpose(2, 1, 0)).astype(bf16)
        xo_h = np.ascontiguousarray(xperm).astype(np.float32)
        gq_all = np.concatenate(
            [np.arange(blk * NT, (blk + 1) * NT) for blk in order]).astype(np.float64)

        # masks: [128, nm*NT]; tile (qi,slot,j,mi): kv rows of that 128-tile
        msk_h = np.zeros((128, max(nm, 1) * NT), dtype=bf16)
        for qi in range(4):
            gq = gq_all[qi * NT:(qi + 1) * NT]
            for (slot, j, mi) in tiles[qi]:
                if mi is None:
                    continue
                k0 = KV_ORDER[slot] * NT + j * 128
                gk = np.arange(k0, k0 + 128).astype(np.float64)
                diff = gk[:, None] - gq[None, :]
                m = np.where(diff <= 0, np.exp(_sigmoid(diff + pbm)), 0.0)
                msk_h[:, mi * NT:(mi + 1) * NT] = m.astype(bf16)

        in_maps.append({
            "xt": xt_h, "xo": xo_h, "wkv": wkv_h, "wql": wql_h,
            "wproj": wproj_h, "msk": msk_h,
        })
    return in_maps


def kernel(x, expand, project, position_bias_mult):
    global LAST_RESULTS
    x = np.asarray(x, dtype=np.float32)
    expand = np.asarray(expand, dtype=np.float32)
    project = np.asarray(project, dtype=np.float32)
    pbm = float(np.asarray(position_bias_mult))

    tiles, nm = _build_sched(pbm)
    in_maps = _prep_inputs(x, expand, project, pbm, tiles, nm)
    nc = _build_nc(tiles, nm)
    res = run_bass_kernel_spmd(nc, in_maps, core_ids=list(range(8)))
    LAST_RESULTS = res

    full = np.empty((B, N, D), dtype=np.float32)
    for c in range(8):
        b, half = c // 2, c % 2
        order = _QORD[half]
        o = res.results[c]["out"]
        for qi in range(4):
            blk = order[qi]
            full[b, blk * NT:(blk + 1) * NT] = o[qi * NT:(qi + 1) * NT]
    return full


# revision 15
# speedup vs baseline: 1.7237x; 1.0196x over previous
"""Trainium2 Bass kernel for nn_AttentionLayer (B=4, N=4096, D=1024) — v2 fp8.

Reference computation:
  nx = layernorm(x)
  h  = nx @ expand                       # [B,N,4352]
  q  = h[:, :128] ; k = h[:, 128:256]
  linear = h[:, 256:2304]; pre_gelu = h[:, 2304:4352]
  gated  = linear * gelu(pre_gelu)       # exact erf gelu
  local  = gated[:, :1024]; v = gated[:, 1024:2048]
  mask[i,j] = j<=i ? sigmoid((j-i)+pbm) : -inf
  attn = softmax(q k^T / sqrt(128) + mask) @ v
  out  = x + concat([local, attn]) @ project

Sharding (8 cores, SPMD): batch b -> core pair (2b, 2b+1); 512-row query
blocks interleave for causal balance (even core owns blocks {0,3,4,7}, odd
{1,2,5,6}).  Each core computes LN + expand for its OWN 2048 rows; k/v are
exchanged via four pairwise fp8 AllGathers (one per 512-row chunk).

v2 changes vs baseline:
  * all heavy matmuls in fp8 e4m3 with DoubleRow perf mode (2 K-chunks per
    pass): expand, local, q/k, AV, project.  Scores (K=128) in fp8 single
    rate.  Scale plan: W*64, x''*1, q*4, k*4, v*16, concat*16; descales are
    folded into ACT scale args and DVE scalar_tensor_tensor constants.
    Simulated end-to-end rel err 1.22e-2 (budget 2e-2).
  * V and K^T live in SBUF for the whole attention phase (loaded once per
    AllGather chunk) instead of being re-DMA'd per query block: removes the
    serialized per-tile DMA issue bottleneck on the sync engine.
  * mask multiply skipped for tiles where exp(sigmoid(j-i+pbm)) == 1 in
    bf16 on BOTH cores (union list keeps the SPMD stream uniform); masked
    tiles get a host-precomputed multiplicative expM tile.
  * batched DMAs (one per x chunk / weight tensor / mask buffer).
  * attention scores for q-block i+1 are emitted between the score and AV
    groups of block i so EXP latency hides under PE work.
"""

import math

import numpy as np
import ml_dtypes

import concourse.bass as bass
import concourse.mybir as mybir
from concourse import bacc
import concourse.tile as tile
from concourse.bass_utils import run_bass_kernel_spmd

BF16 = mybir.dt.bfloat16
FP8 = mybir.dt.float8e4
F32 = mybir.dt.float32
AF = mybir.ActivationFunctionType
DR = mybir.MatmulPerfMode.DoubleRow

B, N, D = 4, 4096, 1024
QK = 128
E = 2048
NT = 512           # row-chunk / matmul free-dim tile
RO = 2048          # own query rows per core
DCH = 8            # 128-deep contraction chunks in D
W2 = 2176          # 128 (q or k) + 1024 (lin half) + 1024 (gel half)

# scales (see fp8_sim2.py)
WS = 64.0          # weight scale for all fp8 weights
QS = 4.0           # q stored scale
KS = 4.0           # k stored scale
VS = 16.0          # v / gated stored scale
CS = 16.0          # concat (local/attn) stored scale == VS
S_E = 1.0 / WS                 # expand psum descale (xs=1)
S_L = 1.0 / (QS * KS * math.sqrt(QK))   # logits psum descale
S_OUT = 1.0 / (CS * WS)        # project psum descale

_QORD = {0: [0, 3, 4, 7], 1: [1, 2, 5, 6]}   # q-block of (half, qi)
KV_ORDER = [0, 3, 4, 7, 1, 2, 5, 6]          # global block of kv slot s
SCHED = {0: [0, 4], 1: [0, 1, 4, 5], 2: [0, 1, 2, 4, 5, 6],
         3: [0, 1, 2, 3, 4, 5, 6, 7]}

LAST_RESULTS = None  # set by kernel(); test harness reads exec_time_ns


def _build_sched(pbm):
    """Per qi: list of (slot, j, mask_idx|None).  mask needed iff some core's
    tile is not entirely in the exp(sigmoid)==1 (bf16) far-below-diag zone."""
    tiles = {}
    nm = 0
    for qi in range(4):
        lst = []
        for slot in SCHED[qi]:
            for j in range(4):
                kv_max = KV_ORDER[slot] * NT + j * 128 + 127
                masked = any(
                    not (kv_max - _QORD[h][qi] * NT + pbm < -6.5)
                    for h in (0, 1))
                if masked:
                    lst.append((slot, j, nm))
                    nm += 1
                else:
                    lst.append((slot, j, None))
        tiles[qi] = lst
    return tiles, nm


def _build_nc(tiles, nm):
    nc = bacc.Bacc(None)

    xt = nc.declare_dram_parameter("xt", [128, DCH, RO], BF16, isOutput=False)
    xo = nc.declare_dram_parameter("xo", [RO, D], F32, isOutput=False)
    wkv = nc.declare_dram_parameter("wkv", [128, DCH, W2], FP8, isOutput=False)
    wql = nc.declare_dram_parameter("wql", [128, DCH, W2], FP8, isOutput=False)
    wproj = nc.declare_dram_parameter("wproj", [128, 16, D], FP8, isOutput=False)
    msk = nc.declare_dram_parameter("msk", [128, max(nm, 1) * NT], BF16,
                                    isOutput=False)
    out = nc.declare_dram_parameter("out", [RO, D], F32, isOutput=True)

    with tile.TileContext(nc) as tc:
        with tc.tile_pool(name="const", bufs=1) as cpool:
            ones128 = cpool.tile([128, 1], BF16)
            nc.vector.memset(ones128[:], 1.0)
            ones1 = cpool.tile([1, 128], BF16)
            nc.vector.memset(ones1[:], 1.0)

            with tc.tile_pool(name="dram", bufs=1, space="DRAM") as dpool:
                kvp = [dpool.tile([128, 4608], FP8, name=f"kvp_{r}")
                       for r in range(4)]
                kv_all = [dpool.tile([2 * 128, 4608], FP8, name=f"kva_{r}")
                          for r in range(4)]

                with tc.tile_pool(name="persist", bufs=1) as ppool:
                    kT_sb = ppool.tile([128, 8, NT], FP8)       # k^T slots
                    qT_sb = ppool.tile([128, 4, NT], FP8)       # q^T chunks
                    v_sb = [ppool.tile([128, 4, 1024], FP8, name=f"vsb_{s}")
                            for s in range(8)]
                    localT_sb = ppool.tile([128, 8, RO], FP8)
                    attnT_sb = ppool.tile([128, 8, RO], FP8)

                    # ---------------- Phase 1: expand (kv part) ----------------
                    with tc.tile_pool(name="wkv_p", bufs=1) as wkvp, \
                         tc.tile_pool(name="wql_p", bufs=1) as wqlp, \
                         tc.tile_pool(name="ex_stream", bufs=2) as estream, \
                         tc.tile_pool(name="ex_work", bufs=3) as ework, \
                         tc.tile_pool(name="st_work", bufs=2) as swork, \
                         tc.tile_pool(name="ex_psum", bufs=5, space="PSUM") as epsum, \
                         tc.tile_pool(name="st_psum", bufs=2, space="PSUM") as spsum:
                        # HAM warmup: dependency-free matmuls trigger the PE
                        # un-throttle (~3.4us activity window) while the first
                        # DMAs are in flight
                        warm_src = ework.tile([1, NT], BF16, tag="warm", bufs=1)
                        nc.vector.memset(warm_src[:], 0.0)
                        warm_ps = epsum.tile([128, NT], F32, tag="mm",
                                             name="warm_ps")
                        for _w in range(12):
                            nc.tensor.matmul(warm_ps[:], ones1[:], warm_src[:],
                                             start=True, stop=True)

                        # x chunk 0 DMA first so stats can start immediately
                        xt_tiles = {}
                        x8_tiles = {}

                        def xt_dma(rch):
                            t = estream.tile([128, DCH, NT], BF16, tag="xt",
                                             bufs=2, name=f"xt_{rch}")
                            nc.sync.dma_start(t[:], xt[:, :, rch * NT:(rch + 1) * NT])
                            xt_tiles[rch] = t

                        xt_dma(0)
                        wkv_sb = wkvp.tile([128, DCH, W2], FP8)
                        nc.sync.dma_start(wkv_sb[:], wkv[:])
                        xt_dma(1)
                        wql_sb = wqlp.tile([128, DCH, W2], FP8)
                        nc.sync.dma_start(wql_sb[:], wql[:])

                        def stats_chain(rch):
                            xts = xt_tiles[rch]
                            mu_ps = spsum.tile([1, NT], F32, tag="stat",
                                               name=f"mu_ps_{rch}")
                            sq_ps = spsum.tile([1, NT], F32, tag="stat",
                                               name=f"sq_ps_{rch}")
                            # accumulate both stats on the PE (psum accumulate)
                            for dch in range(DCH):
                                nc.tensor.matmul(mu_ps[:], ones128[:],
                                                 xts[:, dch:dch + 1, :],
                                                 start=(dch == 0),
                                                 stop=(dch == DCH - 1))
                            for dch in range(DCH):
                                sqt = ework.tile([128, NT], BF16, tag="sq_s",
                                                 bufs=3, name=f"sq_{rch}_{dch}")
                                nc.scalar.activation(sqt[:], xts[:, dch:dch + 1, :],
                                                     AF.Square)
                                nc.tensor.matmul(sq_ps[:], ones128[:], sqt[:],
                                                 start=(dch == 0),
                                                 stop=(dch == DCH - 1))
                            mu = swork.tile([1, NT], F32, tag="st_mu", bufs=1,
                                            name=f"mu_{rch}")
                            e2 = swork.tile([1, NT], F32, tag="st_e2", bufs=1,
                                            name=f"e2_{rch}")
                            scr = swork.tile([1, NT], F32, tag="st_scr", bufs=1,
                                             name=f"scr_{rch}")
                            nc.vector.tensor_scalar_mul(mu[:], mu_ps[:], 1.0 / D)
                            nc.vector.tensor_scalar_mul(e2[:], sq_ps[:], 1.0 / D)
                            nc.vector.tensor_mul(scr[:], mu[:], mu[:])
                            nc.vector.tensor_sub(e2[:], e2[:], scr[:])
                            nc.vector.tensor_scalar_add(e2[:], e2[:], 1e-5)
                            nc.scalar.activation(e2[:], e2[:], AF.Sqrt)
                            nc.vector.reciprocal_approx_fast(scr[:], e2[:])   # rstd
                            nc.vector.scalar_tensor_tensor(
                                mu[:], mu[:], -1.0, scr[:],
                                op0=mybir.AluOpType.mult, op1=mybir.AluOpType.mult)
                            rstd16 = swork.tile([1, NT], BF16, tag="st_r16",
                                                name=f"r16_{rch}")
                            sneg16 = swork.tile([1, NT], BF16, tag="st_s16",
                                                name=f"s16_{rch}")
                            nc.vector.tensor_copy(rstd16[:], scr[:])
                            nc.vector.tensor_copy(sneg16[:], mu[:])
                            return rstd16, sneg16

                        def bcast_chain(rch, rstd16, sneg16):
                            bps = spsum.tile([128, NT], F32, tag="bcast", bufs=1,
                                             name=f"bps_{rch}")
                            nc.tensor.matmul(bps[:], ones1[:], rstd16[:],
                                             start=True, stop=True)
                            rstd_bt = swork.tile([128, NT], BF16, tag="rbt",
                                                 bufs=3, name=f"rbt_{rch}")
                            nc.vector.tensor_copy(rstd_bt[:], bps[:])
                            bps2 = spsum.tile([128, NT], F32, tag="bcast", bufs=1,
                                              name=f"bps2_{rch}")
                            nc.tensor.matmul(bps2[:], ones1[:], sneg16[:],
                                             start=True, stop=True)
                            sneg_bt = swork.tile([128, NT], BF16, tag="sbt",
                                                 bufs=3, name=f"sbt_{rch}")
                            nc.vector.tensor_copy(sneg_bt[:], bps2[:])
                            return rstd_bt, sneg_bt

                        def center_chain(rch, rstd_bt, sneg_bt):
                            xts = xt_tiles[rch]
                            x8 = estream.tile([128, DCH, NT], FP8, tag="x8",
                                              bufs=4, name=f"x8_{rch}")
                            for dch in range(DCH):
                                tmp = ework.tile([128, NT], BF16, tag="ctmp",
                                                 bufs=3, name=f"ct_{rch}_{dch}")
                                nc.vector.tensor_mul(tmp[:], xts[:, dch:dch + 1, :],
                                                     rstd_bt[:])
                                nc.vector.tensor_add(x8[:, dch:dch + 1, :],
                                                     tmp[:], sneg_bt[:])
                            x8_tiles[rch] = x8
                            return x8

                        def v_group(rch, x8, ms):
                            for m in ms:
                                vps = [epsum.tile([128, NT], F32, tag="mm",
                                                  name=f"vps_{rch}_{m}_{_i}")
                                       for _i in range(4)]
                                for pc in range(4):
                                    lhs = x8[:, 2 * pc:2 * pc + 2,
                                             m * 128:(m + 1) * 128]
                                    for vc in range(4):
                                        woff = 128 + vc * NT
                                        nc.tensor.matmul(
                                            vps[vc][:], lhs,
                                            wkv_sb[:, 2 * pc:2 * pc + 2,
                                                   woff:woff + NT],
                                            start=(pc == 0), stop=(pc == 3),
                                            perf_mode=DR)
                                vgel = ework.tile([128, 2 * NT], BF16, tag="vgel")
                                nc.scalar.activation(vgel[:, 0:NT], vps[2][:],
                                                     AF.Gelu, scale=S_E)
                                nc.scalar.activation(vgel[:, NT:2 * NT], vps[3][:],
                                                     AF.Gelu, scale=S_E)
                                vv = ework.tile([128, 2 * NT], FP8, tag="vv",
                                                bufs=4)
                                nc.vector.scalar_tensor_tensor(
                                    vv[:, 0:NT], vps[0][:], S_E * VS,
                                    vgel[:, 0:NT],
                                    op0=mybir.AluOpType.mult,
                                    op1=mybir.AluOpType.mult)
                                nc.vector.scalar_tensor_tensor(
                                    vv[:, NT:2 * NT], vps[1][:], S_E * VS,
                                    vgel[:, NT:2 * NT],
                                    op0=mybir.AluOpType.mult,
                                    op1=mybir.AluOpType.mult)
                                nc.sync.dma_start(
                                    kvp[rch][:, m * 1024:(m + 1) * 1024], vv[:])

                        # chunk pipeline: stats one chunk ahead
                        st0 = stats_chain(0)
                        bt0 = bcast_chain(0, st0[0], st0[1])
                        x8_stash = center_chain(0, bt0[0], bt0[1])
                        for rch in range(4):
                            if rch + 2 <= 3:
                                xt_dma(rch + 2)
                            x8 = x8_stash
                            # k^T (hT layout) -> DRAM for the AllGather
                            kps = epsum.tile([128, NT], F32, tag="mm")
                            for pc in range(4):
                                nc.tensor.matmul(kps[:],
                                                 wkv_sb[:, 2 * pc:2 * pc + 2, 0:128],
                                                 x8[:, 2 * pc:2 * pc + 2, :],
                                                 start=(pc == 0), stop=(pc == 3),
                                                 perf_mode=DR)
                            kout = ework.tile([128, NT], FP8, tag="kout", bufs=2,
                                              name=f"kout_{rch}")
                            nc.vector.tensor_scalar_mul(kout[:], kps[:], S_E * KS)
                            nc.sync.dma_start(kvp[rch][:, 4096:4608], kout[:])
                            v_group(rch, x8, (0, 1))
                            if rch + 1 <= 3:
                                nxt = stats_chain(rch + 1)
                                nbt = bcast_chain(rch + 1, nxt[0], nxt[1])
                                x8_stash = center_chain(rch + 1, nbt[0], nbt[1])
                            v_group(rch, x8, (2, 3))
                            nc.gpsimd.collective_compute(
                                "AllGather",
                                mybir.AluOpType.bypass,
                                replica_groups=[[0, 1], [2, 3], [4, 5], [6, 7]],
                                ins=[kvp[rch].opt()],
                                outs=[kv_all[rch].opt()],
                            )

                        def slot_loads(rch):
                            # v/kT slot loads (own + foreign); deferred to
                            # phase 2 so their AllGather waits never block a
                            # DMA ring that phase-1 traffic is queued on
                            for rank in range(2):
                                slot = rank * 4 + rch
                                nc.sync.dma_start(
                                    v_sb[slot][:],
                                    kv_all[rch][rank * 128:(rank + 1) * 128, 0:4096])
                                nc.sync.dma_start(
                                    kT_sb[:, slot:slot + 1, :],
                                    kv_all[rch][rank * 128:(rank + 1) * 128,
                                                4096:4608])

                        # ---------------- Phase 2: q + local expand ----------------
                        for rch in range(4):
                            slot_loads(rch)
                            x8 = x8_tiles[rch]
                            qps = epsum.tile([128, NT], F32, tag="mm")
                            for pc in range(4):
                                nc.tensor.matmul(qps[:],
                                                 wql_sb[:, 2 * pc:2 * pc + 2, 0:128],
                                                 x8[:, 2 * pc:2 * pc + 2, :],
                                                 start=(pc == 0), stop=(pc == 3),
                                                 perf_mode=DR)
                            nc.vector.tensor_scalar_mul(
                                qT_sb[:, rch:rch + 1, :], qps[:], S_E * QS)
                            for lc in range(8):
                                lps = epsum.tile([128, NT], F32, tag="mm")
                                gps = epsum.tile([128, NT], F32, tag="mm")
                                for pc in range(4):
                                    nc.tensor.matmul(
                                        lps[:],
                                        wql_sb[:, 2 * pc:2 * pc + 2,
                                               128 + lc * 128:256 + lc * 128],
                                        x8[:, 2 * pc:2 * pc + 2, :],
                                        start=(pc == 0), stop=(pc == 3),
                                        perf_mode=DR)
                                for pc in range(4):
                                    nc.tensor.matmul(
                                        gps[:],
                                        wql_sb[:, 2 * pc:2 * pc + 2,
                                               1152 + lc * 128:1280 + lc * 128],
                                        x8[:, 2 * pc:2 * pc + 2, :],
                                        start=(pc == 0), stop=(pc == 3),
                                        perf_mode=DR)
                                lgel = ework.tile([128, NT], BF16, tag="lgel")
                                nc.scalar.activation(lgel[:], gps[:], AF.Gelu,
                                                     scale=S_E)
                                nc.vector.scalar_tensor_tensor(
                                    localT_sb[:, lc:lc + 1,
                                              rch * NT:(rch + 1) * NT],
                                    lps[:], S_E * CS, lgel[:],
                                    op0=mybir.AluOpType.mult,
                                    op1=mybir.AluOpType.mult)

    # ---------------- Phase 3: attention ----------------
                    with tc.tile_pool(name="proj_p", bufs=1) as projp:
                      proj_sb = projp.tile([128, 16, D], FP8)
                      msk_sb = projp.tile([128, max(nm, 1), NT], BF16)
                      # split mask prefetch: first qi-0 tiles, then the rest
                      sp = min(8, nm) if nm else 0
                      if sp:
                          nc.sync.dma_start(msk_sb[:, 0:sp, :], msk[:, 0:sp * NT])
                      nc.sync.dma_start(proj_sb[:], wproj[:])
                      if nm > sp:
                          nc.sync.dma_start(msk_sb[:, sp:nm, :],
                                            msk[:, sp * NT:nm * NT])
                      ones_dr_t = projp.tile([128, 2, 16], FP8)
                      nc.vector.memset(ones_dr_t[:], 1.0)
                      ones_dr = ones_dr_t[:, :, 0:1]   # dim1 step 16B (DR req)
                      with tc.tile_pool(name="psb_p", bufs=2) as psbp, \
                           tc.tile_pool(name="at_work", bufs=2) as awork, \
                           tc.tile_pool(name="at_psum", bufs=2, space="PSUM") as apsum, \
                           tc.tile_pool(name="av_psum", bufs=4, space="PSUM") as avpsum, \
                           tc.tile_pool(name="dn_psum", bufs=2, space="PSUM") as dnpsum:

                        psbs = {}
                        den_pss = {}

                        def score_steps(qi):
                            """One step per kv 128-row tile: pt matmul -> exp
                            (-> mask mul) -> psb; every 2nd tile adds the pair
                            to den via a DoubleRow ones matmul (PE, not DVE)."""
                            tl = tiles[qi]
                            nkr = len(tl)
                            psb = psbp.tile([128, 32, NT], FP8, tag="psb",
                                            name=f"psb_{qi % 2}")
                            den_ps = dnpsum.tile([1, NT], F32, tag="den", bufs=2,
                                                 name=f"den_ps_{qi % 2}")
                            psbs[qi] = psb
                            den_pss[qi] = den_ps

                            def step(ti):
                                slot, j, mi = tl[ti]
                                pt_ps = apsum.tile([128, NT], F32, tag="pt",
                                                   bufs=2, name=f"pt_{qi}_{ti}")
                                nc.tensor.matmul(
                                    pt_ps[:],
                                    kT_sb[:, slot:slot + 1, j * 128:(j + 1) * 128],
                                    qT_sb[:, qi:qi + 1, :],
                                    start=True, stop=True)
                                if mi is None:
                                    nc.scalar.activation(psb[:, ti:ti + 1, :],
                                                         pt_ps[:], AF.Exp,
                                                         scale=S_L)
                                else:
                                    pe = awork.tile([128, NT], BF16, tag="pe",
                                                    bufs=4, name=f"pe_{qi}_{ti}")
                                    nc.scalar.activation(pe[:], pt_ps[:], AF.Exp,
                                                         scale=S_L)
                                    nc.vector.tensor_mul(
                                        psb[:, ti:ti + 1, :], pe[:],
                                        msk_sb[:, mi:mi + 1, :])
                                if ti % 2 == 1:
                                    nc.tensor.matmul(
                                        den_ps[:], ones_dr,
                                        psb[:, ti - 1:ti + 1, :],
                                        start=(ti == 1), stop=(ti == nkr - 1),
                                        perf_mode=DR)

                            return [lambda ti=ti: step(ti) for ti in range(nkr)]

                        def av_steps(qi):
                            """Steps: per (g, pair) 4 DR matmuls; recip chain
                            between g0 and g1; attnT muls after each g."""
                            tl = tiles[qi]
                            nkr = len(tl)
                            steps = []
                            avs_box = {}
                            rd_box = {}

                            def mk_av(g, tp):
                                def go():
                                    if tp == 0:
                                        avs_box[g] = [
                                            avpsum.tile([128, NT], F32, tag="av",
                                                        bufs=4,
                                                        name=f"av{qi}_{g}_{i}")
                                            for i in range(4)]
                                    avs = avs_box[g]
                                    slot, j, _ = tl[2 * tp]
                                    jp = j // 2
                                    for v4 in range(4):
                                        nc.tensor.matmul(
                                            avs[v4][:],
                                            v_sb[slot][:, 2 * jp:2 * jp + 2,
                                                       g * NT + v4 * 128:
                                                       g * NT + (v4 + 1) * 128],
                                            psbs[qi][:, 2 * tp:2 * tp + 2, :],
                                            start=(tp == 0),
                                            stop=(tp == nkr // 2 - 1),
                                            perf_mode=DR)
                                return go

                            def mk_rd():
                                def go():
                                    den = awork.tile([1, NT], F32, tag="den_sb",
                                                     name=f"den_sb_{qi}")
                                    rec = awork.tile([1, NT], F32, tag="rec",
                                                     name=f"rec_{qi}")
                                    rec16 = awork.tile([1, NT], BF16, tag="rec16",
                                                       name=f"rec16_{qi}")
                                    nc.vector.tensor_copy(den[:], den_pss[qi][:])
                                    nc.vector.reciprocal_approx_fast(rec[:], den[:])
                                    nc.vector.tensor_copy(rec16[:], rec[:])
                                    rb_ps = apsum.tile([128, NT], F32, tag="pt",
                                                       bufs=2, name=f"rb_{qi}")
                                    nc.tensor.matmul(rb_ps[:], ones1[:], rec16[:],
                                                     start=True, stop=True)
                                    rd_b = awork.tile([128, NT], BF16, tag="rd_b",
                                                      name=f"rd_b_{qi}")
                                    nc.vector.tensor_copy(rd_b[:], rb_ps[:])
                                    rd_box[0] = rd_b
                                return go

                            def mk_mul(g):
                                def go():
                                    for v4 in range(4):
                                        nc.vector.tensor_mul(
                                            attnT_sb[:, g * 4 + v4:g * 4 + v4 + 1,
                                                     qi * NT:(qi + 1) * NT],
                                            avs_box[g][v4][:], rd_box[0][:])
                                return go

                            for tp in range(nkr // 2):
                                steps.append(mk_av(0, tp))
                            steps.append(mk_rd())
                            steps.append(mk_mul(0))
                            for tp in range(nkr // 2):
                                steps.append(mk_av(1, tp))
                            steps.append(mk_mul(1))
                            return steps

                        # interleave: av(qi) MM stream carries scores(qi+1)
                        # emission so EXP latency hides under PE work
                        for s in score_steps(0):
                            s()
                        for qi in range(4):
                            av = av_steps(qi)
                            sc = score_steps(qi + 1) if qi < 3 else []
                            ns, na = len(sc), len(av)
                            si = 0
                            for ai in range(na):
                                av[ai]()
                                want = (ai + 1) * ns // na
                                while si < want:
                                    sc[si]()
                                    si += 1

                      # ---------------- Phase 4: project + residual ----------------
                      with tc.tile_pool(name="pr_stream", bufs=4) as prstream, \
                           tc.tile_pool(name="pr_psum", bufs=4, space="PSUM") as prpsum:
                          for rt in range(RO // 128):
                              ops = [prpsum.tile([128, NT], F32, tag="out",
                                                 name=f"ops_{rt}_{_i}")
                                     for _i in range(2)]
                              for p in range(8):
                                  if p < 4:
                                      lhs = localT_sb[:, 2 * p:2 * p + 2,
                                                      rt * 128:(rt + 1) * 128]
                                  else:
                                      lhs = attnT_sb[:, 2 * (p - 4):2 * (p - 4) + 2,
                                                     rt * 128:(rt + 1) * 128]
                                  for dc in range(2):
                                      nc.tensor.matmul(
                                          ops[dc][:], lhs,
                                          proj_sb[:, 2 * p:2 * p + 2,
                                                  dc * NT:(dc + 1) * NT],
                                          start=(p == 0), stop=(p == 7),
                                          perf_mode=DR)
                              xo_t = prstream.tile([128, D], F32, tag="xo")
                              nc.sync.dma_start(xo_t[:],
                                                xo[rt * 128:(rt + 1) * 128, :])
                              ot = prstream.tile([128, D], F32, tag="ot")
                              for dc in range(2):
                                  nc.vector.scalar_tensor_tensor(
                                      ot[:, dc * NT:(dc + 1) * NT], ops[dc][:],
                                      S_OUT, xo_t[:, dc * NT:(dc + 1) * NT],
                                      op0=mybir.AluOpType.mult,
                                      op1=mybir.AluOpType.add)
                              nc.sync.dma_start(out[rt * 128:(rt + 1) * 128, :],
                                                ot[:])

    nc.compile()
    return nc


def _sigmoid(x):
    return np.where(x >= 0, 1.0 / (1.0 + np.exp(-np.abs(x))),
                    np.exp(-np.abs(x)) / (1.0 + np.exp(-np.abs(x))))


def _q8(x, scale):
    y = np.clip(np.asarray(x, np.float32) * scale, -240.0, 240.0)
    return np.asarray(y, dtype=ml_dtypes.float8_e4m3)


def _prep_inputs(x, expand, project, pbm, tiles, nm):
    """Build per-core input maps (host-side sharding)."""
    bf16 = ml_dtypes.bfloat16
    wq = expand[:, :QK]
    wk = expand[:, QK:2 * QK]
    lin = expand[:, 2 * QK:2 * QK + E]
    gel = expand[:, 2 * QK + E:]
    # [128, DCH, W2]: [p, dch, col] = W[dch*128+p, colmap]
    wkv_f = np.concatenate([wk, lin[:, D:], gel[:, D:]], axis=1)
    wql_f = np.concatenate([wq, lin[:, :D], gel[:, :D]], axis=1)
    wkv_h = _q8(wkv_f.reshape(DCH, 128, W2).transpose(1, 0, 2), WS)
    wql_h = _q8(wql_f.reshape(DCH, 128, W2).transpose(1, 0, 2), WS)
    wproj_h = _q8(project.reshape(16, 128, D).transpose(1, 0, 2), WS)

    in_maps = []
    for c in range(8):
        b, half = c // 2, c % 2
        order = _QORD[half]
        xb = x[b]
        xperm = np.concatenate([xb[blk * NT:(blk + 1) * NT] for blk in order],
                               axis=0)                      # [2048, 1024]
        # xt[p, dch, n] = xperm[n, dch*128+p]
        xt_h = np.ascontiguousarray(
            xperm.reshape(RO, DCH, 128).transpose(2, 1, 0)).astype(bf16)
        xo_h = np.ascontiguousarray(xperm).astype(np.float32)
        gq_all = np.concatenate(
            [np.arange(blk * NT, (blk + 1) * NT) for blk in order]).astype(np.float64)

        # masks: [128, nm*NT]; tile (qi,slot,j,mi): kv rows of that 128-tile
        msk_h = np.zeros((128, max(nm, 1) * NT), dtype=bf16)
        for qi in range(4):
            gq = gq_all[qi * NT:(qi + 1) * NT]
            for (slot, j, mi) in tiles[qi]:
                if mi is None:
                    continue
                k0 = KV_ORDER[slot] * NT + j * 128
                gk = np.arange(k0, k0 + 128).astype(np.float64)
                diff = gk[:, None] - gq[None, :]
                m = np.where(diff <= 0, np.exp(_sigmoid(diff + pbm)), 0.0)
                msk_h[:, mi * NT:(mi + 1) * NT] = m.astype(bf16)

        in_maps.append({
            "xt": xt_h, "xo": xo_h, "wkv": wkv_h, "wql": wql_h,
            "wproj": wproj_h, "msk": msk_h,
        })
    return in_maps


def kernel(x, expand, project, position_bias_mult):
    global LAST_RESULTS
    x = np.asarray(x, dtype=np.float32)
    expand = np.asarray(expand, dtype=np.float32)
    project = np.asarray(project, dtype=np.float32)
    pbm = float(np.asarray(position_bias_mult))

    tiles, nm = _build_sched(pbm)
    in_maps = _prep_inputs(x, expand, project, pbm, tiles, nm)
    nc = _build_nc(tiles, nm)
    res = run_bass_kernel_spmd(nc, in_maps, core_ids=list(range(8)))
    LAST_RESULTS = res

    full = np.empty((B, N, D), dtype=np.float32)
    for c in range(8):
        b, half = c // 2, c % 2
        order = _QORD[half]
        o = res.results[c]["out"]
        for qi in range(4):
            blk = order[qi]
            full[b, blk * NT:(blk + 1) * NT] = o[qi * NT:(qi + 1) * NT]
    return full
